# revision 10
# baseline (speedup 1.0000x reference)
"""Trainium2 Bass kernel for BackProjectionNet (filtered back-projection style).

Math: for each angle i, the reference broadcasts proj=image[:,i] along x into a
volume, rotates it (bilinear, zero-pad) by phi_i, and accumulates; likewise a
ones-volume into norm; output = obj / (norm + delta).

Because the broadcast volume is constant along x, the rotated sample at output
pixel (x, y) only needs two taps of proj along r:
    obj[b,x,y,z] = sum_i A0(i,x,y)*proj_i[b, Y0(i,x,y), z]
                       + A1(i,x,y)*proj_i[b, Y1(i,x,y), z]
and norm[x,y] is image-independent (host-precomputable from angles alone).

Angle symmetries (linspace(0,360,120)):
  - angle i+60 = angle i + 180: exact r-flip of the sampling position, so the
    projections merge on host:  pm_i = p_i + flip_r(p_{i+60})  (120 -> 60).
  - angle 60-j mirrors angle j through 90 degrees: its backprojection is the
    y-flip of backprojecting with the angle-j tap matrix.  The flip is folded
    into the stationary operand's access pattern (negative strides), so only
    31 tap matrices are stored and everything accumulates into one PSUM bank.

Device mapping (per core, x-rows sharded 16 per core):
  - everything in fp8-e4m3; consecutive angles are paired into DoubleRow
    matmuls (K=2x128, 0.5 cycles per output row -> 4x less PE time than the
    fp32r formulation):
      psum[y,(b,z)] += sum_{h=0,1} W_{pair,h}[r,y].T @ pm_{pair,h}[r,(b,z)]
  - fill phase: 7 x-rows accumulate against chunked weight/pm arrival so the
    PE has work while the ~4.4 MB startup transient streams in; input DMAs
    alternate between the two HWDGE queues (sync/scalar) to double the issue
    rate; tiny non-DR matmuls pad any PE idle so the HAM clock-gate flips to
    2.4 GHz early and stays there.
  - epilogue: multiply by host-precomputed 1/(norm_q+delta) per (x,y) where
    norm_q is derived from the fp8-quantized weights (quantization error in
    the weights then largely cancels in the divide), DMA out.
"""

import numpy as np

B, NA, L, LZ = 2, 120, 128, 128
NM = 60            # merged angle count
NCORES = 8
XPC = L // NCORES  # x rows per core
NW = 31            # stored weight angles (0..30); 31..59 via mirror symmetry
FILLX = 7          # x-rows accumulated during the fill phase
# weight slot chunks for fill x-rows and the pairs gated on each.
# Chunks are sized >=240KB: each transfer pays ~0.65us queue-engine issue
# plus ~1.3us completion-receipt latency, so small chunks gate consumers
# on overhead rather than bytes.
WCH = [(0, 16), (16, 31)]
# pm slot chunks
PCH = [(0, 16), (16, 30), (30, 46), (46, 60)]
# fill phases: (pair range, wt chunk idx needed, pm chunk idx needed)
PHASES = [
    (range(0, 8), 0, 0),
    (range(8, 15), 1, 1),
    (range(15, 23), None, 2),
    (range(23, 30), None, 3),
]
PRE_DUMMIES = 11   # back-to-back DR warmers bridging program start to the
                   # first input chunks' availability (~10.4us); the real
                   # matmul stream then continues gaplessly so the HAM clock
                   # gate flips to 2.4 GHz ~3.4us after the first dummy and
                   # never re-throttles (post-flip idle tolerance ~1.7us)


def _host_tables(angles):
    """Replicate reference fp32 tap math: banded lhsT weights (fp8-quantized)
    for merged angles 0..30 + 1/(norm_q+delta) built from those fp8 weights."""
    import ml_dtypes

    dt = np.float32
    f8 = ml_dtypes.float8_e4m3
    phis = (-np.deg2rad(angles)).astype(dt)
    cx = dt((L - 1) / 2.0)
    xs = np.arange(L, dtype=dt) - cx
    X, Y = np.meshgrid(xs, xs, indexing="ij")
    W = np.zeros((NW, L, L, L), dt)  # [j, x, r, y] for angles 0..NW-1
    xg = np.broadcast_to(np.arange(L)[:, None], (L, L)).ravel()
    yg = np.broadcast_to(np.arange(L)[None, :], (L, L)).ravel()
    one = dt(1.0)
    for i in range(NW):
        c = np.float32(np.cos(phis[i]))
        s = np.float32(np.sin(phis[i]))
        sx = c * X + s * Y + cx
        sy = -s * X + c * Y + cx
        x0 = np.floor(sx)
        y0 = np.floor(sy)
        wx = (sx - x0).astype(dt)
        wy = (sy - y0).astype(dt)
        x0i = x0.astype(np.int64)
        y0i = y0.astype(np.int64)
        vx0 = ((x0i >= 0) & (x0i < L)).astype(dt)
        vx1 = ((x0i + 1 >= 0) & (x0i + 1 < L)).astype(dt)
        vy0 = ((y0i >= 0) & (y0i < L)).astype(dt)
        vy1 = ((y0i + 1 >= 0) & (y0i + 1 < L)).astype(dt)
        g = (one - wx) * vx0 + wx * vx1
        A0 = ((one - wy) * vy0 * g).astype(dt)
        A1 = (wy * vy1 * g).astype(dt)
        Y0 = np.clip(y0i, 0, L - 1).ravel()
        Y1 = np.clip(y0i + 1, 0, L - 1).ravel()
        np.add.at(W[i], (xg, Y0, yg), A0.ravel())
        np.add.at(W[i], (xg, Y1, yg), A1.ravel())
    W8 = W.astype(f8)
    # norm consistent with what the device computes: angles 0..29 use slots
    # 0..29 straight, angles 30..59 use slots 30..1 with the y axis flipped,
    # and each merged projection carries two of the 120 original angles.
    cs8 = W8.astype(dt).sum(axis=2)  # [j, x, y]
    norm_q = 2.0 * (cs8[0:NM // 2].sum(axis=0)
                    + cs8[1:NM // 2 + 1].sum(axis=0)[:, ::-1])
    inv = (one / (norm_q + dt(1e-11))).astype(dt)
    return W8, inv


def _merge_ok(angles):
    a = np.asarray(angles, np.float64)
    return a.shape == (NA,) and np.allclose(a[NM:], a[:NM] + 180.0, atol=1e-4)


def _cpu_fallback(image, angles):
    """Straight numpy evaluation of the tap formula (safety net only)."""
    dt = np.float32
    phis = (-np.deg2rad(angles)).astype(dt)
    cx = dt((L - 1) / 2.0)
    xs = np.arange(L, dtype=dt) - cx
    X, Y = np.meshgrid(xs, xs, indexing="ij")
    obj = np.zeros((B, L, L, LZ), dt)
    norm = np.zeros((L, L), dt)
    one = dt(1.0)
    for i in range(len(angles)):
        c = np.float32(np.cos(phis[i]))
        s = np.float32(np.sin(phis[i]))
        sx = c * X + s * Y + cx
        sy = -s * X + c * Y + cx
        x0i = np.floor(sx).astype(np.int64)
        y0i = np.floor(sy).astype(np.int64)
        wx = (sx - np.floor(sx)).astype(dt)
        wy = (sy - np.floor(sy)).astype(dt)
        vx0 = ((x0i >= 0) & (x0i < L)).astype(dt)
        vx1 = ((x0i + 1 >= 0) & (x0i + 1 < L)).astype(dt)
        vy0 = ((y0i >= 0) & (y0i < L)).astype(dt)
        vy1 = ((y0i + 1 >= 0) & (y0i + 1 < L)).astype(dt)
        norm += ((one - wx) * (one - wy) * vx0 * vy0
                 + (one - wx) * wy * vx0 * vy1
                 + wx * (one - wy) * vx1 * vy0
                 + wx * wy * vx1 * vy1)
        g = (one - wx) * vx0 + wx * vx1
        A0 = (one - wy) * vy0 * g
        A1 = wy * vy1 * g
        Y0 = np.clip(y0i, 0, L - 1)
        Y1 = np.clip(y0i + 1, 0, L - 1)
        p = image[:, i]  # [B, L, LZ]
        obj += A0[None, :, :, None] * p[:, Y0, :] + A1[None, :, :, None] * p[:, Y1, :]
    return obj / (norm + dt(1e-11))[None, :, :, None]


def _build_bass():
    import concourse.bacc as bacc
    import concourse.mybir as mybir
    import concourse.tile as tile

    f32 = mybir.dt.float32
    bf16 = mybir.dt.bfloat16
    f8 = mybir.dt.float8e4
    DR = mybir.MatmulPerfMode.DoubleRow

    nc = bacc.Bacc(None, target_bir_lowering=False, debug=False)
    with tile.TileContext(nc) as tc:
        with tc.tile_pool(name="dram", bufs=1, space="DRAM") as dram:
            pmat = dram.tile([L, NM, B, LZ], f8, kind="ExternalInput",
                             name="pm", uniquify=False)
            wts = dram.tile([XPC, L, NW, L], f8, kind="ExternalInput",
                            name="wts", uniquify=False)
            invn = dram.tile([L, XPC], f32, kind="ExternalInput",
                             name="invn", uniquify=False)
            # [x, y, b, z]: host transposes back to [b, x, y, z]; keeps each
            # (x, y) write a contiguous 512B run so output DMAs stay cheap
            outd = dram.tile([XPC, L, B, LZ], bf16, kind="ExternalOutput",
                             name="out", uniquify=False)

            with (
                tc.tile_pool(name="pm_pool", bufs=1) as pm_pool,
                tc.tile_pool(name="wt_pool", bufs=10) as wt_pool,
                tc.tile_pool(name="misc", bufs=1) as misc_pool,
                tc.tile_pool(name="stage_pool", bufs=1) as stage_pool,
                tc.tile_pool(name="warm_pool", bufs=1) as warm_pool,
                tc.tile_pool(name="wps_pool", bufs=1, space="PSUM") as wps_pool,
                tc.tile_pool(name="psum", bufs=7, space="PSUM") as psum_pool,
            ):
                pm = pm_pool.tile([L, NM, B, LZ], f8)
                invn_sb = misc_pool.tile([L, XPC], f32)
                stage = stage_pool.tile([L, XPC, B, LZ], bf16)

                # PE warm-up block: output is never read; scratch operands
                # are zeroed because uninitialized SBUF can hold NaN/Inf fp8
                # patterns, the only run-varying state in the program.
                wlhs = warm_pool.tile([L, 2, L], f8)
                wrhs = warm_pool.tile([L, 2, B * LZ], f8)
                wps = wps_pool.tile([L, B * LZ], f32, tag="warm")
                nc.vector.memset(wlhs[:], 0)
                nc.vector.memset(wrhs[:], 0)

                for _ in range(PRE_DUMMIES):
                    nc.tensor.matmul(out=wps[:], lhsT=wlhs[:], rhs=wrhs[:],
                                     start=True, stop=True, perf_mode=DR,
                                     skip_group_check=True)

                # Input DMAs alternate across the two HWDGE queue engines
                # (sync, scalar) — each dma_start costs ~0.6us on its queue
                # engine and queues FIFO per engine, so alternating doubles
                # both the issue rate and the number of in-flight rings.
                qs = [nc.sync, nc.scalar]
                qi = [0]

                def dma(out, in_):
                    qs[qi[0] % 2].dma_start(out=out, in_=in_)
                    qi[0] += 1

                nc.gpsimd.dma_start(out=invn_sb[:], in_=invn[:])

                wtf = [wt_pool.tile([L, NW, L], f8, tag="wt", name=f"wt{x}")
                       for x in range(FILLX)]
                # transfers in exact PE consumption order: each phase runs
                # x-row 0 first, so that row's weight chunk and the phase's
                # pm chunk must land before the other rows' chunks
                def wdma(x, ci):
                    s0, s1 = WCH[ci]
                    dma(wtf[x][:, s0:s1], wts[x, :, s0:s1])

                def pdma(ci):
                    p0, p1 = PCH[ci]
                    dma(pm[:, p0:p1], pmat[:, p0:p1])

                wdma(0, 0)
                pdma(0)
                for x in range(1, FILLX):
                    wdma(x, 0)
                wdma(0, 1)
                pdma(1)
                for x in range(1, FILLX):
                    wdma(x, 1)
                pdma(2)
                pdma(3)

                psf = [psum_pool.tile([L, B * LZ], f32, tag="ps",
                                      name=f"ps{x}") for x in range(FILLX)]

                def mm_pair(ps, wt, p, start=False, stop=False):
                    if p < 15:  # angles 0..29: slot pairs straight
                        nc.tensor.matmul(
                            out=ps[:], lhsT=wt[:, 2 * p:2 * p + 2, :],
                            rhs=pm[:, 2 * p:2 * p + 2],
                            start=start, stop=stop, perf_mode=DR,
                        )
                    else:  # angles 30..59: mirrored slots, y-flip in the AP
                        k = p - 15
                        hi = 30 - 2 * k
                        nc.tensor.matmul(
                            out=ps[:], lhsT=wt[:, hi:hi - 2:-1, ::-1],
                            rhs=pm[:, 30 + 2 * k:32 + 2 * k],
                            start=start, stop=stop, perf_mode=DR,
                        )

                for plist, _, _ in PHASES:
                    for x in range(FILLX):
                        for p in plist:
                            mm_pair(psf[x], wtf[x], p, start=(p == 0),
                                    stop=(p == 29))

                def epilogue(x, ps):
                    nc.vector.tensor_scalar_mul(
                        out=stage[:, x].rearrange("y b z -> y (b z)"),
                        in0=ps[:],
                        scalar1=invn_sb[:, x:x + 1],
                    )

                for x in range(FILLX):
                    epilogue(x, psf[x])
                nc.gpsimd.dma_start(
                    out=outd[0:4].rearrange("x y b z -> y x (b z)"),
                    in_=stage[:, 0:4].rearrange("y x b z -> y x (b z)"),
                )

                # steady loop: one x-row at a time; weight DMAs keep
                # alternating queues; outputs ride the idle SWDGE (gpsimd)
                # ring except the final row (HWDGE for minimum tail latency)
                out_groups = {7: (4, 8), 11: (8, 12), 13: (12, 14)}
                for x in range(FILLX, XPC):
                    wt = wt_pool.tile([L, NW, L], f8, tag="wt", name=f"wt{x}")
                    dma(wt[:], wts[x])
                    ps = psum_pool.tile([L, B * LZ], f32, tag="ps",
                                        name=f"ps{x}")
                    for p in range(30):
                        mm_pair(ps, wt, p, start=(p == 0), stop=(p == 29))
                    if x < XPC - 1:
                        epilogue(x, ps)
                    else:
                        # final row: per-batch epilogue so each half's DMA
                        # issues as soon as it is ready, on parallel queues
                        for b in range(B):
                            nc.vector.tensor_scalar_mul(
                                out=stage[:, x, b],
                                in0=ps[:, b * LZ:(b + 1) * LZ],
                                scalar1=invn_sb[:, x:x + 1],
                            )
                            eng = nc.scalar if b == 0 else nc.sync
                            eng.dma_start(
                                out=outd[x:x + 1, :, b].rearrange(
                                    "x y z -> y x z"),
                                in_=stage[:, x:x + 1, b],
                            )
                    if x in out_groups:
                        g0, g1 = out_groups[x]
                        nc.gpsimd.dma_start(
                            out=outd[g0:g1].rearrange("x y b z -> y x (b z)"),
                            in_=stage[:, g0:g1].rearrange(
                                "y x b z -> y x (b z)"),
                        )
                nc.scalar.dma_start(
                    out=outd[XPC - 2:XPC - 1].rearrange("x y b z -> y x (b z)"),
                    in_=stage[:, XPC - 2:XPC - 1].rearrange(
                        "y x b z -> y x (b z)"),
                )

    nc.compile()
    return nc


_BASS_CACHE = {}


def _make_in_maps(image, W8, inv):
    import ml_dtypes

    f8 = ml_dtypes.float8_e4m3
    # merged projections, pre-transposed for a contiguous device DMA
    pm = image[:, :NM] + image[:, NM:, ::-1, :]          # [B, NM, L, LZ]
    pm8 = np.ascontiguousarray(pm.transpose(2, 1, 0, 3)).astype(f8)
    in_maps = []
    for k in range(NCORES):
        xsl = slice(XPC * k, XPC * (k + 1))
        wk = np.ascontiguousarray(W8[:, xsl].transpose(1, 2, 0, 3))
        in_maps.append({
            "pm": pm8,
            "wts": wk,  # [XPC, r, NW, y] fp8
            "invn": np.ascontiguousarray(inv[xsl].T),  # [y, x] f32
        })
    return in_maps


def kernel(image, angles):
    image = np.ascontiguousarray(np.asarray(image, np.float32))
    angles = np.asarray(angles, np.float32)
    if not _merge_ok(angles):
        return _cpu_fallback(image, angles)

    from concourse.bass_utils import run_bass_kernel_spmd

    W8, inv = _host_tables(angles)

    if "nc" not in _BASS_CACHE:
        _BASS_CACHE["nc"] = _build_bass()
    nc = _BASS_CACHE["nc"]

    in_maps = _make_in_maps(image, W8, inv)

    res = run_bass_kernel_spmd(nc, in_maps, core_ids=list(range(NCORES)))
    # out is [XPC, L, B, LZ] per core -> [L, L, B, LZ] -> [B, Lx, Ly, Lz]
    out = np.concatenate([r["out"] for r in res.results], axis=0)
    out = out.transpose(2, 0, 1, 3)
    return np.ascontiguousarray(out.astype(np.float32))


# revision 16
# speedup vs baseline: 1.0535x; 1.0535x over previous
"""Trainium2 Bass kernel for BackProjectionNet (filtered back-projection style).

Math: for each angle i, the reference broadcasts proj=image[:,i] along x into a
volume, rotates it (bilinear, zero-pad) by phi_i, and accumulates; likewise a
ones-volume into norm; output = obj / (norm + delta).

Because the broadcast volume is constant along x, the rotated sample at output
pixel (x, y) only needs two taps of proj along r:
    obj[b,x,y,z] = sum_i A0(i,x,y)*proj_i[b, Y0(i,x,y), z]
                       + A1(i,x,y)*proj_i[b, Y1(i,x,y), z]
and norm[x,y] is image-independent (host-precomputable from angles alone).

Angle symmetries (linspace(0,360,120)):
  - angle i+60 = angle i + 180: exact r-flip of the sampling position, so the
    projections merge on host:  pm_i = p_i + flip_r(p_{i+60})  (120 -> 60).
  - angle 60-j mirrors angle j through 90 degrees: its backprojection is the
    y-flip of backprojecting with the angle-j tap matrix.  The flip is folded
    into the stationary operand's access pattern (negative strides), so only
    31 tap matrices are stored.

Device mapping: fp8-e4m3 everywhere; consecutive angles pair into DoubleRow
matmuls psum[y,(b,z)] += W_pair[r,y].T @ pm_pair[r,(b,z)].  The whole kernel
runs in the PE's 2x-row-tiling mode (64-row tiles at partitions 0 and 64,
which run CONCURRENTLY): each pair's tap band in r either crosses r=64 (two
tile matmuls, same cost as one full-K matmul) or fits one half (ONE tile
matmul - half cost).  Bands at edge x-rows all fall in the same half, so x is
sharded INTERLEAVED (core k owns x = 8s+k) and complementary rows (s, 15-s)
are leapfrogged: row A's low-tile chain runs while row B's high-tile chain
runs, then they swap.  Both chains of a row share ONE psum bank (sequential,
never simultaneous - separated by >=3 intervening matmuls).  A wide 6-row
fill phase amortizes the 2MB pm stream while chunked weights land; a ~4.3us
block of warm-up matmuls at program start flips the HAM clock gate to 2.4GHz
before real work begins and the gapless stream keeps it there.

Epilogue per row: multiply by host-precomputed 1/(norm_q+delta) per (x,y)
where norm_q is derived from the fp8-quantized weights (quantization error
in the weights then largely cancels in the divide), DMA out.
"""

import numpy as np

B, NA, L, LZ = 2, 120, 128, 128
NM = 60            # merged angle count
NCORES = 8
XPC = L // NCORES  # x rows per core
NW = 31            # stored weight angles (0..30); 31..59 via mirror symmetry

# interleaved sharding: core k owns x = 8s+k; s is the row index below.
# fill rows (central: bands straddle r=64 so tiling can't help them; run
# full-128 mode, one psum bank each, amortizing the pm stream across all 7),
# then leapfrog pairs of complementary edge rows in 64-row-tile mode, and a
# final lone full-K row for a minimal-latency tail.
FILL_S = [4, 5, 6, 7, 8, 9, 10]
PAIR_S = [(3, 12), (2, 13), (1, 14), (0, 15)]
LAST_S = 11
SLOT_ORDER = FILL_S + [s for ab in PAIR_S for s in ab] + [LAST_S]

# fill-phase chunking: weight slot chunks / pm slot chunks / phase gating
WCH = [(0, 8), (8, 16), (16, 31)]
PCH = [(0, 8), (8, 16), (16, 30), (30, 46), (46, 60)]
PHASES = [(range(0, 4), 0, 0), (range(4, 8), 1, 1), (range(8, 15), 2, 2),
          (range(15, 23), None, 3), (range(23, 30), None, 4)]

PRE_DUMMIES = 18   # back-to-back DR warmers (~4.3us): the HAM clock gate
                   # flips to 2.4 GHz ~4.2-5us after a gapless stream begins,
                   # so force the flip during the dummy block; afterwards
                   # idle gaps under ~1.7us do not re-throttle


def _host_tables(angles):
    """Replicate reference fp32 tap math: banded lhsT weights (fp8-quantized)
    for merged angles 0..30 + 1/(norm_q+delta) built from those fp8 weights."""
    import ml_dtypes

    dt = np.float32
    f8 = ml_dtypes.float8_e4m3
    phis = (-np.deg2rad(angles)).astype(dt)
    cx = dt((L - 1) / 2.0)
    xs = np.arange(L, dtype=dt) - cx
    X, Y = np.meshgrid(xs, xs, indexing="ij")
    W = np.zeros((NW, L, L, L), dt)  # [j, x, r, y] for angles 0..NW-1
    xg = np.broadcast_to(np.arange(L)[:, None], (L, L)).ravel()
    yg = np.broadcast_to(np.arange(L)[None, :], (L, L)).ravel()
    one = dt(1.0)
    for i in range(NW):
        c = np.float32(np.cos(phis[i]))
        s = np.float32(np.sin(phis[i]))
        sx = c * X + s * Y + cx
        sy = -s * X + c * Y + cx
        x0 = np.floor(sx)
        y0 = np.floor(sy)
        wx = (sx - x0).astype(dt)
        wy = (sy - y0).astype(dt)
        x0i = x0.astype(np.int64)
        y0i = y0.astype(np.int64)
        vx0 = ((x0i >= 0) & (x0i < L)).astype(dt)
        vx1 = ((x0i + 1 >= 0) & (x0i + 1 < L)).astype(dt)
        vy0 = ((y0i >= 0) & (y0i < L)).astype(dt)
        vy1 = ((y0i + 1 >= 0) & (y0i + 1 < L)).astype(dt)
        g = (one - wx) * vx0 + wx * vx1
        A0 = ((one - wy) * vy0 * g).astype(dt)
        A1 = (wy * vy1 * g).astype(dt)
        Y0 = np.clip(y0i, 0, L - 1).ravel()
        Y1 = np.clip(y0i + 1, 0, L - 1).ravel()
        np.add.at(W[i], (xg, Y0, yg), A0.ravel())
        np.add.at(W[i], (xg, Y1, yg), A1.ravel())
    W8 = W.astype(f8)
    # norm consistent with what the device computes: angles 0..29 use slots
    # 0..29 straight, angles 30..59 use slots 30..1 with the y axis flipped,
    # and each merged projection carries two of the 120 original angles.
    cs8 = W8.astype(dt).sum(axis=2)  # [j, x, y]
    norm_q = 2.0 * (cs8[0:NM // 2].sum(axis=0)
                    + cs8[1:NM // 2 + 1].sum(axis=0)[:, ::-1])
    inv = (one / (norm_q + dt(1e-11))).astype(dt)
    return W8, inv


def _merge_ok(angles):
    a = np.asarray(angles, np.float64)
    return a.shape == (NA,) and np.allclose(a[NM:], a[:NM] + 180.0, atol=1e-4)


def _pair_slots(p):
    if p < 15:
        return 2 * p, 2 * p + 1
    k = p - 15
    return 30 - 2 * k, 29 - 2 * k


def _tile_classes():
    """(s, p) -> 'low' | 'high' | 'cross': where the pair's tap band sits in
    r, unioned over the 8 interleaved x rows (8s..8s+7) and both slots, for
    the linspace angles the kernel is specialized to."""
    phis = -np.deg2rad(np.linspace(0.0, 360.0, NA, endpoint=False))[:NM]
    cx = (L - 1) / 2.0
    ys = np.arange(L) - cx
    cls = {}
    for s in range(16):
        for p in range(30):
            lo, hi = L, 0
            for x in range(8 * s, 8 * s + 8):
                X = x - cx
                for j in _pair_slots(p):
                    c, sn = np.cos(phis[j]), np.sin(phis[j])
                    sy = -sn * X + c * ys + cx
                    y0 = np.floor(sy).astype(int)
                    taps = np.concatenate([y0, y0 + 1])
                    taps = taps[(taps >= 0) & (taps <= L - 1)]
                    if taps.size:
                        lo = min(lo, taps.min())
                        hi = max(hi, taps.max() + 1)
            cls[(s, p)] = ("low" if hi <= 64 else
                           ("high" if lo >= 64 else "cross"))
    return cls


def _cpu_fallback(image, angles):
    """Straight numpy evaluation of the tap formula (safety net only)."""
    dt = np.float32
    phis = (-np.deg2rad(angles)).astype(dt)
    cx = dt((L - 1) / 2.0)
    xs = np.arange(L, dtype=dt) - cx
    X, Y = np.meshgrid(xs, xs, indexing="ij")
    obj = np.zeros((B, L, L, LZ), dt)
    norm = np.zeros((L, L), dt)
    one = dt(1.0)
    for i in range(len(angles)):
        c = np.float32(np.cos(phis[i]))
        s = np.float32(np.sin(phis[i]))
        sx = c * X + s * Y + cx
        sy = -s * X + c * Y + cx
        x0i = np.floor(sx).astype(np.int64)
        y0i = np.floor(sy).astype(np.int64)
        wx = (sx - np.floor(sx)).astype(dt)
        wy = (sy - np.floor(sy)).astype(dt)
        vx0 = ((x0i >= 0) & (x0i < L)).astype(dt)
        vx1 = ((x0i + 1 >= 0) & (x0i + 1 < L)).astype(dt)
        vy0 = ((y0i >= 0) & (y0i < L)).astype(dt)
        vy1 = ((y0i + 1 >= 0) & (y0i + 1 < L)).astype(dt)
        norm += ((one - wx) * (one - wy) * vx0 * vy0
                 + (one - wx) * wy * vx0 * vy1
                 + wx * (one - wy) * vx1 * vy0
                 + wx * wy * vx1 * vy1)
        g = (one - wx) * vx0 + wx * vx1
        A0 = (one - wy) * vy0 * g
        A1 = wy * vy1 * g
        Y0 = np.clip(y0i, 0, L - 1)
        Y1 = np.clip(y0i + 1, 0, L - 1)
        p = image[:, i]  # [B, L, LZ]
        obj += A0[None, :, :, None] * p[:, Y0, :] + A1[None, :, :, None] * p[:, Y1, :]
    return obj / (norm + dt(1e-11))[None, :, :, None]


def _build_bass():
    import concourse.bacc as bacc
    import concourse.mybir as mybir
    import concourse.tile as tile

    f32 = mybir.dt.float32
    bf16 = mybir.dt.bfloat16
    f8 = mybir.dt.float8e4
    DR = mybir.MatmulPerfMode.DoubleRow
    Copy = mybir.ActivationFunctionType.Copy

    cls = _tile_classes()

    nc = bacc.Bacc(None, target_bir_lowering=False, debug=False)
    with tile.TileContext(nc) as tc:
        with tc.tile_pool(name="dram", bufs=1, space="DRAM") as dram:
            pmat = dram.tile([L, NM, B, LZ], f8, kind="ExternalInput",
                             name="pm", uniquify=False)
            wts = dram.tile([XPC, L, NW, L], f8, kind="ExternalInput",
                            name="wts", uniquify=False)
            invn = dram.tile([L, XPC], f32, kind="ExternalInput",
                             name="invn", uniquify=False)
            # [pos, y, b, z]: host reorders back to [b, x, y, z]; keeps each
            # (x, y) write a contiguous 512B run so output DMAs stay cheap
            outd = dram.tile([XPC, L, B, LZ], bf16, kind="ExternalOutput",
                             name="out", uniquify=False)

            with (
                tc.tile_pool(name="pm_pool", bufs=1) as pm_pool,
                tc.tile_pool(name="wt_pool", bufs=10) as wt_pool,
                tc.tile_pool(name="misc", bufs=1) as misc_pool,
                tc.tile_pool(name="stage_pool", bufs=1) as stage_pool,
                tc.tile_pool(name="warm_pool", bufs=1) as warm_pool,
                tc.tile_pool(name="epi", bufs=2) as epi_pool,
                tc.tile_pool(name="wps_pool", bufs=1, space="PSUM") as wpp,
                tc.tile_pool(name="psum", bufs=7, space="PSUM") as psum_pool,
            ):
                pm = pm_pool.tile([L, NM, B, LZ], f8)
                invn_sb = misc_pool.tile([L, XPC], f32)
                stage = stage_pool.tile([L, XPC, B, LZ], bf16)

                # PE warm-up block (full-128 mode; one mode switch into the
                # 64-row-tiled stream after).  Output is never read; scratch
                # operands are zeroed because uninitialized SBUF can hold
                # NaN/Inf fp8 patterns, the only run-varying program state.
                wlhs = warm_pool.tile([L, 2, L], f8)
                wrhs = warm_pool.tile([L, 2, B * LZ], f8)
                wps = wpp.tile([L, B * LZ], f32, tag="warm")
                nc.vector.memset(wlhs[:], 0)
                nc.vector.memset(wrhs[:], 0)
                for _ in range(PRE_DUMMIES):
                    nc.tensor.matmul(out=wps[:], lhsT=wlhs[:], rhs=wrhs[:],
                                     start=True, stop=True, perf_mode=DR,
                                     skip_group_check=True)

                # Input DMAs alternate across the two HWDGE queue engines
                # (sync, scalar): each dma_start costs ~0.6us on its queue
                # engine and queues FIFO per engine, so alternating doubles
                # the issue rate and overlaps two rings.
                qs = [nc.sync, nc.scalar]
                qi = [0]

                def dma(out, in_):
                    qs[qi[0] % 2].dma_start(out=out, in_=in_)
                    qi[0] += 1

                nc.gpsimd.dma_start(out=invn_sb[:], in_=invn[:])

                # row state, keyed by row index s
                row_wt = {}
                row_pos = {s: i for i, s in enumerate(SLOT_ORDER)}

                def mm_full(ps, wt, p, start, stop):
                    """Full-128 DR matmul for pair p (fill + last row)."""
                    if p < 15:
                        nc.tensor.matmul(
                            out=ps[:], lhsT=wt[:, 2 * p:2 * p + 2, :],
                            rhs=pm[:, 2 * p:2 * p + 2],
                            start=start, stop=stop, perf_mode=DR,
                        )
                    else:
                        k = p - 15
                        hi = 30 - 2 * k
                        nc.tensor.matmul(
                            out=ps[:], lhsT=wt[:, hi:hi - 2:-1, ::-1],
                            rhs=pm[:, 30 + 2 * k:32 + 2 * k],
                            start=start, stop=stop, perf_mode=DR,
                        )

                def mm_half(ps, wt, p, half, start, stop):
                    """64-row-tile DR matmul: pair p, K-window half."""
                    h0, h1 = 64 * half, 64 * half + 64
                    if p < 15:
                        nc.tensor.matmul(
                            out=ps[:], lhsT=wt[h0:h1, 2 * p:2 * p + 2, :],
                            rhs=pm[h0:h1, 2 * p:2 * p + 2],
                            start=start, stop=stop, perf_mode=DR,
                            skip_group_check=True,
                        )
                    else:
                        k = p - 15
                        hi = 30 - 2 * k
                        nc.tensor.matmul(
                            out=ps[:], lhsT=wt[h0:h1, hi:hi - 2:-1, ::-1],
                            rhs=pm[h0:h1, 30 + 2 * k:32 + 2 * k],
                            start=start, stop=stop, perf_mode=DR,
                            skip_group_check=True,
                        )

                def epilogue1(s, ps):
                    pos = row_pos[s]
                    nc.vector.tensor_scalar_mul(
                        out=stage[:, pos].rearrange("y b z -> y (b z)"),
                        in0=ps[:],
                        scalar1=invn_sb[:, pos:pos + 1],
                    )

                # ---- fill: 7 central rows, full-128 mode, chunk-phased
                for s in FILL_S:
                    row_wt[s] = wt_pool.tile([L, NW, L], f8, tag="wt",
                                             name=f"wt{s}")
                psf = {s: psum_pool.tile([L, B * LZ], f32, tag="ps",
                                         name=f"ps{s}") for s in FILL_S}

                def wdma(s, ci):
                    c0, c1 = WCH[ci]
                    dma(row_wt[s][:, c0:c1], wts[row_pos[s], :, c0:c1])

                def pdma(ci):
                    p0, p1 = PCH[ci]
                    dma(pm[:, p0:p1], pmat[:, p0:p1])

                wdma(FILL_S[0], 0)
                pdma(0)
                for s in FILL_S[1:]:
                    wdma(s, 0)
                wdma(FILL_S[0], 1)
                pdma(1)
                for s in FILL_S[1:]:
                    wdma(s, 1)
                wdma(FILL_S[0], 2)
                pdma(2)
                for s in FILL_S[1:]:
                    wdma(s, 2)
                pdma(3)
                pdma(4)

                for plist, _, _ in PHASES:
                    for s in FILL_S:
                        for p in plist:
                            mm_full(psf[s], row_wt[s], p, start=(p == 0),
                                    stop=(p == 29))

                for s in FILL_S:
                    epilogue1(s, psf[s])
                nc.gpsimd.dma_start(
                    out=outd[0:4].rearrange("x y b z -> y x (b z)"),
                    in_=stage[:, 0:4].rearrange("y x b z -> y x (b z)"),
                )

                # ---- steady: leapfrog pairs of complementary edge rows in
                # 64-row-tile mode.  Each row gets TWO psum banks: one
                # written only by the (0,0) tile, one only by the (64,0)
                # tile (a psum bank cannot be shared between row tiles).
                # The low-tile stream runs row A then row B while the
                # high-tile stream runs row B then row A - both tiles stay
                # busy even though each row's band-fitting pairs all sit in
                # the same half.
                for pi, (ra, rb) in enumerate(PAIR_S):
                    psl = {}
                    psh = {}
                    for s in (ra, rb):
                        row_wt[s] = wt_pool.tile([L, NW, L], f8, tag="wt",
                                                 name=f"wt{s}")
                        dma(row_wt[s][:], wts[row_pos[s]])
                        psl[s] = psum_pool.tile([L, B * LZ], f32, tag="ps",
                                                name=f"psl{s}")
                        psh[s] = psum_pool.tile([L, B * LZ], f32, tag="ps",
                                                name=f"psh{s}")
                    t0q = ([(ra, p) for p in range(30)
                            if cls[(ra, p)] != "high"]
                           + [(rb, p) for p in range(30)
                              if cls[(rb, p)] != "high"])
                    t8q = ([(rb, p) for p in range(30)
                            if cls[(rb, p)] != "low"]
                           + [(ra, p) for p in range(30)
                              if cls[(ra, p)] != "low"])
                    n0 = {}
                    n8 = {}
                    for s, _ in t0q:
                        n0[s] = n0.get(s, 0) + 1
                    for s, _ in t8q:
                        n8[s] = n8.get(s, 0) + 1
                    c0 = {}
                    c8 = {}
                    done = {ra: 0, rb: 0}

                    def emit(q, idx, half, cn, nn, pss):
                        s, p = q[idx]
                        cn[s] = cn.get(s, 0) + 1
                        mm_half(pss[s], row_wt[s], p, half,
                                start=(cn[s] == 1), stop=(cn[s] == nn[s]))
                        if cn[s] == nn[s]:
                            done[s] += 1
                            if done[s] == 2:
                                epilogue2(s, psl[s], psh[s])

                    def epilogue2(s, pl, ph):
                        pos = row_pos[s]
                        sa = epi_pool.tile([L, B * LZ], f32, tag="sa",
                                           name=f"sa{s}")
                        sb = epi_pool.tile([L, B * LZ], f32, tag="sb",
                                           name=f"sb{s}")
                        sc = invn_sb[:, pos:pos + 1]
                        nc.scalar.activation(out=sa[:], in_=pl[:], func=Copy,
                                             scale=sc)
                        nc.vector.tensor_scalar_mul(out=sb[:], in0=ph[:],
                                                    scalar1=sc)
                        nc.vector.tensor_tensor(
                            out=stage[:, pos].rearrange("y b z -> y (b z)"),
                            in0=sa[:], in1=sb[:],
                            op=mybir.AluOpType.add)

                    for i in range(max(len(t0q), len(t8q))):
                        if i < len(t0q):
                            emit(t0q, i, 0, c0, n0, psl)
                        if i < len(t8q):
                            emit(t8q, i, 1, c8, n8, psh)

                    # output DMA: batch finished positions on the idle
                    # SWDGE ring
                    if pi == 1:
                        nc.gpsimd.dma_start(
                            out=outd[4:8].rearrange("x y b z -> y x (b z)"),
                            in_=stage[:, 4:8].rearrange(
                                "y x b z -> y x (b z)"),
                        )
                    elif pi == 3:
                        nc.gpsimd.dma_start(
                            out=outd[8:12].rearrange("x y b z -> y x (b z)"),
                            in_=stage[:, 8:12].rearrange(
                                "y x b z -> y x (b z)"),
                        )

                # ---- last row: full-128 mode again, minimal tail: per-b
                # epilogue + output DMAs on parallel HWDGE queues
                s = LAST_S
                pos = row_pos[s]
                wt = wt_pool.tile([L, NW, L], f8, tag="wt", name=f"wt{s}")
                dma(wt[:], wts[pos])
                ps = psum_pool.tile([L, B * LZ], f32, tag="ps", name=f"ps{s}")
                for p in range(30):
                    mm_full(ps, wt, p, start=(p == 0), stop=(p == 29))
                nc.scalar.dma_start(
                    out=outd[12:15].rearrange("x y b z -> y x (b z)"),
                    in_=stage[:, 12:15].rearrange("y x b z -> y x (b z)"),
                )
                for b in range(B):
                    nc.vector.tensor_scalar_mul(
                        out=stage[:, pos, b],
                        in0=ps[:, b * LZ:(b + 1) * LZ],
                        scalar1=invn_sb[:, pos:pos + 1],
                    )
                    eng = nc.sync if b == 0 else nc.scalar
                    eng.dma_start(
                        out=outd[pos:pos + 1, :, b].rearrange(
                            "x y z -> y x z"),
                        in_=stage[:, pos:pos + 1, b],
                    )

    nc.compile()
    return nc


_BASS_CACHE = {}


def _make_in_maps(image, W8, inv):
    import ml_dtypes

    f8 = ml_dtypes.float8_e4m3
    # merged projections, pre-transposed for a contiguous device DMA
    pm = image[:, :NM] + image[:, NM:, ::-1, :]          # [B, NM, L, LZ]
    pm8 = np.ascontiguousarray(pm.transpose(2, 1, 0, 3)).astype(f8)
    in_maps = []
    for k in range(NCORES):
        xlist = [8 * s + k for s in SLOT_ORDER]
        wk = np.ascontiguousarray(W8[:, xlist].transpose(1, 2, 0, 3))
        in_maps.append({
            "pm": pm8,
            "wts": wk,  # [pos, r, NW, y] fp8
            "invn": np.ascontiguousarray(inv[xlist].T),  # [y, pos] f32
        })
    return in_maps


def kernel(image, angles):
    image = np.ascontiguousarray(np.asarray(image, np.float32))
    angles = np.asarray(angles, np.float32)
    if not _merge_ok(angles):
        return _cpu_fallback(image, angles)

    from concourse.bass_utils import run_bass_kernel_spmd

    W8, inv = _host_tables(angles)

    if "nc" not in _BASS_CACHE:
        _BASS_CACHE["nc"] = _build_bass()
    nc = _BASS_CACHE["nc"]

    in_maps = _make_in_maps(image, W8, inv)

    res = run_bass_kernel_spmd(nc, in_maps, core_ids=list(range(NCORES)))
    # per-core out is [pos, y, b, z]; scatter rows back to x = 8s+k
    full = np.empty((L, L, B, LZ), np.float32)
    for k in range(NCORES):
        xlist = [8 * s + k for s in SLOT_ORDER]
        full[xlist] = res.results[k]["out"].astype(np.float32)
    return np.ascontiguousarray(full.transpose(2, 0, 1, 3))
